# revision 1
# baseline (speedup 1.0000x reference)
"""Pairwise Euclidean distance kernel for Trainium2 (8 NeuronCores, SPMD).

Problem: mapping [8192, 256] f32 -> out [8192, 8192] f32 where
out[i, j] = ||mapping[i] - mapping[j]||_2, via the GEMM identity
d2 = ||x_i||^2 + ||x_j||^2 - 2 <x_i, x_j>.

Sharding: one 1024-row block of the output per core; every core keeps the
full mapping replicated (the rhs of the gram spans all 8192 columns). To
keep a single SPMD program with static addressing, each core's inputs are
rotated by c*1024 (rows of the natural layout / columns of the transposed
layout) so its own rows always sit first; the host un-rotates each core's
output columns afterwards.

Per-core on-device pipeline (~122 us, DMA-bound at ~96% duty: 32 MB output
+ 8 MB input at ~350 GB/s/core; [128, 1024] output chunks with 4 PSUM
buffers keep the in-order PE dense through the ramp):
  - inputs: mt [256, 8192] f16 (x^T, rotated), nat [8192, 256] f16 (x,
    rotated), eye [128, 128] f32 (transpose identity)
  - matmul dtype f16: the PE multiplies f16 exactly into f32 PSUM, so the
    only error vs the f32 reference is the f16 input rounding (~2e-4
    relative; scale-relative absmax ~8e-4, dominated by the i==j block).
  - sq_j = sum_k x~[j,k]^2 in f32 from the *same* f16-rounded values the
    gram uses, so the diagonal cancels to ~1e-4: squares on ACT (plain
    Square per 8-tile group), 3D-AP reduce on DVE, per 2048-column pair.
  - -0.5*sq_j is split hi/lo into two f16 rows (exact to ~2^-22), built by
    PE-transposing the [128, 16] per-pair slabs and flattening to [2, 2048]
    with a strided DMA; a K=2 rank-1 matmul with an all-ones stationary
    operand folds it into the PSUM accumulation: psum = gram - 0.5*sq_j.
  - ACT computes the whole epilogue in one op per [128, 1024] chunk:
    sqrt(-2*psum + sq_i) with per-partition bias sq_i, reading PSUM.
    d2 only goes negative (fp rounding) in the i==j block, so a [128, 128]
    tensor_scalar_min clamp (psum <= 0.5*sq_i) precedes the sqrt there.
  - schedule: chunk-outer loop, pair-0 sq chain emitted first at high
    priority (the first rank-1 blocks the in-order PE until its sq_flat
    lands), both PSUM slots pre-filled with sq-independent k-matmuls for
    runway, later pairs' sq interleaved into the chunk stream.

Hardware pitfalls encountered (this container's TRN2 + neuronxcc build):
  - InstTensorTensorReduce (fused DVE square+reduce) and ACT Square with
    accum_out both crash the device (NRT_EXEC_UNIT_UNRECOVERABLE); use
    plain Square + separate reduce_sum instead.
  - ACT Sqrt on negative inputs yields NaN (CoreSim asserts); clamp first.
"""

import sys

try:
    import concourse.bass as _probe  # noqa: F401
except ImportError:
    sys.path.insert(0, "/opt/trn_rl_repo")

import numpy as np

import concourse.bacc as bacc
import concourse.mybir as mybir
from concourse import tile
from concourse.bass_utils import run_bass_kernel_spmd

N = 8192          # number of points
D = 256           # feature dim
NCORES = 8
RPC = N // NCORES  # 1024 rows per core
RT = RPC // 128    # 8 row-tiles per core
JCHUNK = 1024      # output chunk width (2 PSUM banks)
NJC = N // JCHUNK  # 8 chunks
NSUB = JCHUNK // 512  # 2 matmul sub-tiles per chunk
PAIRW = 2048       # sq pair width (2 chunks per pair)
NPAIR = N // PAIRW
NGRP = 8           # sq reduction groups (8 tiles of 128 rows each)

F16 = mybir.dt.float16
F32 = mybir.dt.float32


def _build_nc(repeats=1, loop_n=None, stage_bufs=4, work_bufs=2):
    nc = bacc.Bacc(None, target_bir_lowering=False)
    mt_d = nc.dram_tensor("mt", [D, N], F16, kind="ExternalInput")
    nat_d = nc.dram_tensor("nat", [N, D], F16, kind="ExternalInput")
    eye_d = nc.dram_tensor("eye", [128, 128], F32, kind="ExternalInput")
    out_d = nc.dram_tensor("out", [RPC, N], F32, kind="ExternalOutput")

    with tile.TileContext(nc) as tc:
        with (
            tc.tile_pool(name="big", bufs=1) as big,
            tc.tile_pool(name="work", bufs=work_bufs) as work,
            tc.tile_pool(name="stage", bufs=stage_bufs) as stage_pool,
            tc.tile_pool(name="ps", bufs=4, space="PSUM") as psum,
        ):
            if loop_n is not None:
                with tc.For_i(0, loop_n, 1):
                    _emit_body(nc, tc, big, work, stage_pool, psum,
                               mt_d, nat_d, eye_d, out_d)
            else:
                for _rep in range(repeats):
                    _emit_body(nc, tc, big, work, stage_pool, psum,
                               mt_d, nat_d, eye_d, out_d)

    nc.compile()
    return nc


def _emit_body(nc, tc, big, work, stage_pool, psum, mt_d, nat_d, eye_d, out_d):
    # --- persistent SBUF tensors; mt loaded in 2048-column chunks so the
    # first main-loop chunk only depends on the first slice ---
    mt0 = big.tile([128, N], F16, tag="mt0")
    mt1 = big.tile([128, N], F16, tag="mt1")
    eye = big.tile([128, 128], F32, tag="eye")
    ones2 = big.tile([2, 128], F16, tag="ones2")
    # per-pair sq tensors: a single shared tile would create false
    # WAR/RAW couplings (later pairs write other slices while every chunk
    # reads its bias / rank-1 row), serializing the pipeline
    sqp = []
    sqf = []
    for _p in range(NPAIR):
        sqp_t = big.tile([128, 16], F32, tag=f"sqp{_p}")
        sqp.append(sqp_t)
        sqf_t = big.tile([2, PAIRW], F16, tag=f"sqf{_p}")
        sqf.append(sqf_t)

    half_own = big.tile([128, 8], F32, tag="half_own")
    nat_g = nat_d.rearrange("(g t p) d -> g p t d", g=NGRP, p=128)

    # nat-group input tiles all resident at once so the loads can be issued
    # as soon as the DMA pool has capacity
    gts = {}
    for g in range(NGRP):
        gt_slot = big.tile([128, 8, 256], F16, tag=f"natg{g}")
        gts[g] = gt_slot
    nc.sync.dma_start(gts[0][:], nat_g[0])
    nc.sync.dma_start(gts[1][:], nat_g[1])
    nc.sync.dma_start(mt0[:, 0:PAIRW], mt_d[0:128, 0:PAIRW])
    nc.sync.dma_start(mt1[:, 0:PAIRW], mt_d[128:256, 0:PAIRW])
    nc.sync.dma_start(eye[:], eye_d[:])

    def emit_loads(stage):
        # bulk loads for pair/chunk `stage+1`, issued after the pair-0 chain
        # so its small flatten DMAs aren't queued behind megabytes of input
        g0 = 2 + 2 * stage
        nc.sync.dma_start(gts[g0][:], nat_g[g0])
        nc.sync.dma_start(gts[g0 + 1][:], nat_g[g0 + 1])
        j1 = (stage + 1) * PAIRW
        nc.sync.dma_start(mt0[:, j1:j1 + PAIRW], mt_d[0:128, j1:j1 + PAIRW])
        nc.sync.dma_start(mt1[:, j1:j1 + PAIRW], mt_d[128:256, j1:j1 + PAIRW])

    def emit_sq_reduce(pair):
        # sq for j in [pair*2048, (pair+1)*2048): nat groups 2p, 2p+1 ->
        # sqp[pair] [128, 16] -> -0.5 hi/lo f16 slices
        for g in (2 * pair, 2 * pair + 1):
            gt = gts[g]
            gl = g - 2 * pair
            # square on ACT (plain Square, no accum - the fused/accum DVE and
            # ACT variants crash this hardware), reduce on DVE: splits the sq
            # work across both engines and keeps the DVE queue shallow
            msq = work.tile([128, 8, 256], F32, tag="msq")
            nc.scalar.activation(msq[:], gt[:],
                                 mybir.ActivationFunctionType.Square)
            nc.vector.reduce_sum(
                sqp[pair][:, gl * 8:(gl + 1) * 8].unsqueeze(2),
                msq[:],
                axis=mybir.AxisListType.X,
            )
        sl = sqp[pair][:, 0:16]
        mh32 = work.tile([128, 16], F32, tag=f"mh32_{pair}")
        nc.vector.tensor_scalar_mul(mh32[:], sl, -0.5)
        hi16 = work.tile([128, 16], F16, tag=f"hi16_{pair}")
        nc.vector.tensor_copy(hi16[:], mh32[:])
        hi32 = work.tile([128, 16], F32, tag=f"hi32_{pair}")
        nc.vector.tensor_copy(hi32[:], hi16[:])
        lo32 = work.tile([128, 16], F32, tag=f"lo32_{pair}")
        nc.vector.tensor_sub(lo32[:], mh32[:], hi32[:])
        if pair == 0:
            nc.vector.tensor_scalar_mul(half_own[:], sqp[0][:, 0:8], 0.5)
        return mh32, lo32

    def emit_sq_flatten(pair, mh32, lo32):
        # transpose [128, 16] -> [16, 128] on PE, flatten into sq_flat; kept
        # separate so the in-order PE only meets these after the DVE chain
        # has had time to produce mh32/lo32
        for row, src in ((0, mh32), (1, lo32)):
            pt = psum.tile([16, 128], F32, tag="ps")
            nc.tensor.transpose(pt[:], src[:], eye[:])
            st = work.tile([16, 128], F16, tag="sqT")
            nc.vector.tensor_copy(st[:], pt[:])
            nc.sync.dma_start(
                sqf[pair][row:row + 1, :].rearrange("o (t i) -> o t i", t=16),
                st[:],
            )

    # pair-0 sq chain first (the first rank-1 matmul blocks the in-order PE
    # stream until sq_flat[:, 0:2048] lands); high priority so the scheduler
    # does not interleave later pairs' DVE work into this chain
    nc.vector.memset(ones2[:], 1.0)
    with tc.high_priority():
        emit_sq_flatten(0, *emit_sq_reduce(0))
    emit_loads(0)
    emit_loads(1)
    emit_loads(2)

    # --- main loop: chunk-outer so chunk 0 starts as soon as its sq slice
    # and mt slice are resident ---
    for jc in range(NJC):
        nxt = None
        def emit_kmms(ps, r):
            lhs0 = mt0[:, r * 128:(r + 1) * 128]
            lhs1 = mt1[:, r * 128:(r + 1) * 128]
            for s in range(NSUB):
                j0 = jc * JCHUNK + s * 512
                o = ps[:, s * 512:(s + 1) * 512]
                nc.tensor.matmul(o, lhs0, mt0[:, j0:j0 + 512],
                                 start=True, stop=False)
                nc.tensor.matmul(o, lhs1, mt1[:, j0:j0 + 512],
                                 start=False, stop=False)

        def emit_rank1(ps):
            half = (jc % 2) * JCHUNK
            for s in range(NSUB):
                o = ps[:, s * 512:(s + 1) * 512]
                nc.tensor.matmul(
                    o, ones2[:],
                    sqf[jc // 2][:, half + s * 512:half + (s + 1) * 512],
                    start=False, stop=True)

        def emit_tail(ps, r):
            out_t = stage_pool.tile([128, JCHUNK], F32, tag="stage")
            bias = sqp[0][:, r:r + 1]
            if jc == 0:
                # d2 can only go negative (fp rounding) in the i==j block,
                # and ACT Sqrt requires inputs >= 0: clamp psum <= 0.5*sq_i
                # there (so -2*psum + sq_i >= 0) before the sqrt
                dg = ps[:, r * 128:(r + 1) * 128]
                nc.vector.tensor_scalar_min(dg, dg, half_own[:, r:r + 1])
            nc.scalar.activation(
                out_t[:], ps[:],
                mybir.ActivationFunctionType.Sqrt,
                bias=bias, scale=-2.0,
            )
            nc.sync.dma_start(
                out_d[r * 128:(r + 1) * 128,
                      jc * JCHUNK:(jc + 1) * JCHUNK],
                out_t[:],
            )

        if jc == 0:
            # fill both psum slots with sq-independent k-matmuls first so
            # the in-order PE has runway while the sq chain completes
            ps0 = psum.tile([128, JCHUNK], F32, tag="ps")
            emit_kmms(ps0, 0)
            ps1 = psum.tile([128, JCHUNK], F32, tag="ps")
            emit_kmms(ps1, 1)
            emit_rank1(ps0)
            emit_tail(ps0, 0)
            emit_rank1(ps1)
            emit_tail(ps1, 1)
            start_r = 2
        else:
            start_r = 0
        pair_nxt = jc // 2 + 1
        prep_pair = (jc % 2 == 0) and pair_nxt < NPAIR
        for r in range(start_r, RT):
            if r == start_r and prep_pair:
                nxt = emit_sq_reduce(pair_nxt)
            if r == start_r + 1 and nxt is not None:
                emit_sq_flatten(pair_nxt, *nxt)

            ps = psum.tile([128, JCHUNK], F32, tag="ps")
            emit_kmms(ps, r)
            emit_rank1(ps)
            emit_tail(ps, r)


_NC_CACHE = None


def _get_nc():
    global _NC_CACHE
    if _NC_CACHE is None:
        _NC_CACHE = _build_nc()
    return _NC_CACHE


def kernel(mapping: np.ndarray, **_kwargs) -> np.ndarray:
    mapping = np.asarray(mapping, dtype=np.float32)
    assert mapping.shape == (N, D)
    xh = mapping.astype(np.float16)
    eye = np.eye(128, dtype=np.float32)

    in_maps = []
    for c in range(NCORES):
        natc = np.ascontiguousarray(np.roll(xh, -c * RPC, axis=0))
        mtc = np.ascontiguousarray(natc.T)
        in_maps.append({"mt": mtc, "nat": natc, "eye": eye})

    nc = _get_nc()
    res = run_bass_kernel_spmd(nc, in_maps, core_ids=list(range(NCORES)))

    out = np.empty((N, N), dtype=np.float32)
    for c in range(NCORES):
        out[c * RPC:(c + 1) * RPC] = np.roll(res.results[c]["out"], c * RPC, axis=1)
    return out


if __name__ == "__main__":
    rng = np.random.default_rng(0)
    x = rng.standard_normal((N, D)).astype(np.float32)
    o = kernel(mapping=x)
    print("out", o.shape, o.dtype, "sample", o[0, :4], "diag", np.abs(np.diag(o)).max())



# revision 4
# speedup vs baseline: 1.1226x; 1.1226x over previous
"""Pairwise Euclidean distance kernel for Trainium2 (8 NeuronCores, SPMD).

Problem: mapping [8192, 256] f32 -> out [8192, 8192] f32 where
out[i, j] = ||mapping[i] - mapping[j]||_2, via the GEMM identity
d2 = ||x_i||^2 + ||x_j||^2 - 2 <x_i, x_j>.

Sharding: one 1024-row block of the output per core; every core keeps the
full mapping replicated (the rhs of the gram spans all 8192 columns). To
keep a single SPMD program with static addressing, each core's inputs are
rotated by c*1024 (rows of the natural layout / columns of the transposed
layout) so its own rows always sit first; the host un-rotates each core's
output columns afterwards.

Per-core on-device pipeline (~122 us, DMA-bound at ~96% duty: 32 MB output
+ 8 MB input at ~350 GB/s/core; [128, 1024] output chunks with 4 PSUM
buffers keep the in-order PE dense through the ramp):
  - inputs: mt [256, 8192] f16 (x^T, rotated), nat [8192, 256] f16 (x,
    rotated), eye [128, 128] f32 (transpose identity)
  - matmul dtype f16: the PE multiplies f16 exactly into f32 PSUM, so the
    only error vs the f32 reference is the f16 input rounding (~2e-4
    relative; scale-relative absmax ~8e-4, dominated by the i==j block).
  - sq_j = sum_k x~[j,k]^2 in f32 from the *same* f16-rounded values the
    gram uses, so the diagonal cancels to ~1e-4: squares on ACT (plain
    Square per 8-tile group), 3D-AP reduce on DVE, per 2048-column pair.
  - -0.5*sq_j is split hi/lo into two f16 rows (exact to ~2^-22), built by
    PE-transposing the [128, 16] per-pair slabs and flattening to [2, 2048]
    with a strided DMA; a K=2 rank-1 matmul with an all-ones stationary
    operand folds it into the PSUM accumulation: psum = gram - 0.5*sq_j.
  - ACT computes the whole epilogue in one op per [128, 1024] chunk:
    sqrt(-2*psum + sq_i) with per-partition bias sq_i, reading PSUM.
    d2 only goes negative (fp rounding) in the i==j block, so a [128, 128]
    tensor_scalar_min clamp (psum <= 0.5*sq_i) precedes the sqrt there.
  - schedule: chunk-outer loop, pair-0 sq chain emitted first at high
    priority (the first rank-1 blocks the in-order PE until its sq_flat
    lands), both PSUM slots pre-filled with sq-independent k-matmuls for
    runway, later pairs' sq interleaved into the chunk stream.

Hardware pitfalls encountered (this container's TRN2 + neuronxcc build):
  - InstTensorTensorReduce (fused DVE square+reduce) and ACT Square with
    accum_out both crash the device (NRT_EXEC_UNIT_UNRECOVERABLE); use
    plain Square + separate reduce_sum instead.
  - ACT Sqrt on negative inputs yields NaN (CoreSim asserts); clamp first.
"""

import sys

try:
    import concourse.bass as _probe  # noqa: F401
except ImportError:
    sys.path.insert(0, "/opt/trn_rl_repo")

import numpy as np

import concourse.bacc as bacc
import concourse.mybir as mybir
from concourse import tile
from concourse.bass_utils import run_bass_kernel_spmd

N = 8192          # number of points
D = 256           # feature dim
NCORES = 8
RPC = N // NCORES  # 1024 rows per core
RT = RPC // 128    # 8 row-tiles per core
JCHUNK = 1024      # output chunk width (2 PSUM banks)
NJC = N // JCHUNK  # 8 chunks
NSUB = JCHUNK // 512  # 2 matmul sub-tiles per chunk
PAIRW = 2048       # sq pair width (2 chunks per pair)
NPAIR = N // PAIRW
NGRP = 8           # sq reduction groups (8 tiles of 128 rows each)

F16 = mybir.dt.float16
F32 = mybir.dt.float32


def _build_nc(repeats=1, loop_n=None, stage_bufs=4, work_bufs=2):
    nc = bacc.Bacc(None, target_bir_lowering=False)
    mt_d = nc.dram_tensor("mt", [D, N], F16, kind="ExternalInput")
    nat_d = nc.dram_tensor("nat", [N, D], F16, kind="ExternalInput")
    eye_d = nc.dram_tensor("eye", [128, 128], F32, kind="ExternalInput")
    out_d = nc.dram_tensor("out", [RPC, N], F16, kind="ExternalOutput")

    with tile.TileContext(nc) as tc:
        with (
            tc.tile_pool(name="big", bufs=1) as big,
            tc.tile_pool(name="work", bufs=work_bufs) as work,
            tc.tile_pool(name="stage", bufs=stage_bufs) as stage_pool,
            tc.tile_pool(name="ps", bufs=4, space="PSUM") as psum,
        ):
            if loop_n is not None:
                with tc.For_i(0, loop_n, 1):
                    _emit_body(nc, tc, big, work, stage_pool, psum,
                               mt_d, nat_d, eye_d, out_d)
            else:
                for _rep in range(repeats):
                    _emit_body(nc, tc, big, work, stage_pool, psum,
                               mt_d, nat_d, eye_d, out_d)

    nc.compile()
    return nc


def _emit_body(nc, tc, big, work, stage_pool, psum, mt_d, nat_d, eye_d, out_d):
    # --- persistent SBUF tensors; mt loaded in 2048-column chunks so the
    # first main-loop chunk only depends on the first slice ---
    mt0 = big.tile([128, N], F16, tag="mt0")
    mt1 = big.tile([128, N], F16, tag="mt1")
    eye = big.tile([128, 128], F32, tag="eye")
    ones2 = big.tile([2, 128], F16, tag="ones2")
    # per-pair sq tensors: a single shared tile would create false
    # WAR/RAW couplings (later pairs write other slices while every chunk
    # reads its bias / rank-1 row), serializing the pipeline
    sqp = []
    sqf = []
    for _p in range(NPAIR):
        sqp_t = big.tile([128, 16], F32, tag=f"sqp{_p}")
        sqp.append(sqp_t)
        sqf_t = big.tile([2, PAIRW], F16, tag=f"sqf{_p}")
        sqf.append(sqf_t)

    half_own = big.tile([128, 8], F32, tag="half_own")
    nat_g = nat_d.rearrange("(g t p) d -> g p t d", g=NGRP, p=128)

    # nat-group input tiles all resident at once so the loads can be issued
    # as soon as the DMA pool has capacity
    gts = {}
    for g in range(NGRP):
        gt_slot = big.tile([128, 8, 256], F16, tag=f"natg{g}")
        gts[g] = gt_slot
    nc.sync.dma_start(gts[0][:], nat_g[0])
    nc.sync.dma_start(gts[1][:], nat_g[1])
    nc.sync.dma_start(mt0[:, 0:PAIRW], mt_d[0:128, 0:PAIRW])
    nc.sync.dma_start(mt1[:, 0:PAIRW], mt_d[128:256, 0:PAIRW])
    nc.sync.dma_start(eye[:], eye_d[:])

    def emit_loads(stage):
        # bulk loads for pair/chunk `stage+1`, issued after the pair-0 chain
        # so its small flatten DMAs aren't queued behind megabytes of input
        g0 = 2 + 2 * stage
        nc.sync.dma_start(gts[g0][:], nat_g[g0])
        nc.sync.dma_start(gts[g0 + 1][:], nat_g[g0 + 1])
        j1 = (stage + 1) * PAIRW
        nc.sync.dma_start(mt0[:, j1:j1 + PAIRW], mt_d[0:128, j1:j1 + PAIRW])
        nc.sync.dma_start(mt1[:, j1:j1 + PAIRW], mt_d[128:256, j1:j1 + PAIRW])

    def emit_sq_reduce(pair):
        # sq for j in [pair*2048, (pair+1)*2048): nat groups 2p, 2p+1 ->
        # sqp[pair] [128, 16] -> -0.5 hi/lo f16 slices
        for g in (2 * pair, 2 * pair + 1):
            gt = gts[g]
            gl = g - 2 * pair
            # square on ACT (plain Square, no accum - the fused/accum DVE and
            # ACT variants crash this hardware), reduce on DVE: splits the sq
            # work across both engines and keeps the DVE queue shallow
            msq = work.tile([128, 8, 256], F32, tag="msq")
            nc.scalar.activation(msq[:], gt[:],
                                 mybir.ActivationFunctionType.Square)
            nc.vector.reduce_sum(
                sqp[pair][:, gl * 8:(gl + 1) * 8].unsqueeze(2),
                msq[:],
                axis=mybir.AxisListType.X,
            )
        sl = sqp[pair][:, 0:16]
        mh32 = work.tile([128, 16], F32, tag=f"mh32_{pair}")
        nc.vector.tensor_scalar_mul(mh32[:], sl, -0.5)
        hi16 = work.tile([128, 16], F16, tag=f"hi16_{pair}")
        nc.vector.tensor_copy(hi16[:], mh32[:])
        hi32 = work.tile([128, 16], F32, tag=f"hi32_{pair}")
        nc.vector.tensor_copy(hi32[:], hi16[:])
        lo32 = work.tile([128, 16], F32, tag=f"lo32_{pair}")
        nc.vector.tensor_sub(lo32[:], mh32[:], hi32[:])
        if pair == 0:
            nc.vector.tensor_scalar_mul(half_own[:], sqp[0][:, 0:8], 0.5)
        return mh32, lo32

    def emit_sq_flatten(pair, mh32, lo32):
        # transpose [128, 16] -> [16, 128] on PE, flatten into sq_flat; kept
        # separate so the in-order PE only meets these after the DVE chain
        # has had time to produce mh32/lo32
        for row, src in ((0, mh32), (1, lo32)):
            pt = psum.tile([16, 128], F32, tag="ps")
            nc.tensor.transpose(pt[:], src[:], eye[:])
            st = work.tile([16, 128], F16, tag="sqT")
            nc.vector.tensor_copy(st[:], pt[:])
            nc.sync.dma_start(
                sqf[pair][row:row + 1, :].rearrange("o (t i) -> o t i", t=16),
                st[:],
            )

    # pair-0 sq chain first (the first rank-1 matmul blocks the in-order PE
    # stream until sq_flat[:, 0:2048] lands); high priority so the scheduler
    # does not interleave later pairs' DVE work into this chain
    nc.vector.memset(ones2[:], 1.0)
    with tc.high_priority():
        emit_sq_flatten(0, *emit_sq_reduce(0))
    emit_loads(0)
    emit_loads(1)
    emit_loads(2)

    # --- main loop: chunk-outer so chunk 0 starts as soon as its sq slice
    # and mt slice are resident ---
    for jc in range(NJC):
        nxt = None
        def emit_kmms(ps, r):
            lhs0 = mt0[:, r * 128:(r + 1) * 128]
            lhs1 = mt1[:, r * 128:(r + 1) * 128]
            for s in range(NSUB):
                j0 = jc * JCHUNK + s * 512
                o = ps[:, s * 512:(s + 1) * 512]
                nc.tensor.matmul(o, lhs0, mt0[:, j0:j0 + 512],
                                 start=True, stop=False)
                nc.tensor.matmul(o, lhs1, mt1[:, j0:j0 + 512],
                                 start=False, stop=False)

        def emit_rank1(ps):
            half = (jc % 2) * JCHUNK
            for s in range(NSUB):
                o = ps[:, s * 512:(s + 1) * 512]
                nc.tensor.matmul(
                    o, ones2[:],
                    sqf[jc // 2][:, half + s * 512:half + (s + 1) * 512],
                    start=False, stop=True)

        def emit_tail(ps, r):
            out_t = stage_pool.tile([128, JCHUNK], F16, tag="stage")
            bias = sqp[0][:, r:r + 1]
            if jc == 0:
                # d2 can only go negative (fp rounding) in the i==j block,
                # and ACT Sqrt requires inputs >= 0: clamp psum <= 0.5*sq_i
                # there (so -2*psum + sq_i >= 0) before the sqrt
                dg = ps[:, r * 128:(r + 1) * 128]
                nc.vector.tensor_scalar_min(dg, dg, half_own[:, r:r + 1])
            nc.scalar.activation(
                out_t[:], ps[:],
                mybir.ActivationFunctionType.Sqrt,
                bias=bias, scale=-2.0,
            )
            nc.sync.dma_start(
                out_d[r * 128:(r + 1) * 128,
                      jc * JCHUNK:(jc + 1) * JCHUNK],
                out_t[:],
            )

        if jc == 0:
            # fill both psum slots with sq-independent k-matmuls first so
            # the in-order PE has runway while the sq chain completes
            ps0 = psum.tile([128, JCHUNK], F32, tag="ps")
            emit_kmms(ps0, 0)
            ps1 = psum.tile([128, JCHUNK], F32, tag="ps")
            emit_kmms(ps1, 1)
            emit_rank1(ps0)
            emit_tail(ps0, 0)
            emit_rank1(ps1)
            emit_tail(ps1, 1)
            start_r = 2
        else:
            start_r = 0
        pair_nxt = jc // 2 + 1
        prep_pair = (jc % 2 == 0) and pair_nxt < NPAIR
        for r in range(start_r, RT):
            if r == start_r and prep_pair:
                nxt = emit_sq_reduce(pair_nxt)
            if r == start_r + 1 and nxt is not None:
                emit_sq_flatten(pair_nxt, *nxt)

            ps = psum.tile([128, JCHUNK], F32, tag="ps")
            emit_kmms(ps, r)
            emit_rank1(ps)
            emit_tail(ps, r)


_NC_CACHE = None


def _get_nc():
    global _NC_CACHE
    if _NC_CACHE is None:
        _NC_CACHE = _build_nc()
    return _NC_CACHE


def prep_inputs(mapping: np.ndarray) -> list:
    xh = mapping.astype(np.float16)
    eye = np.eye(128, dtype=np.float32)
    in_maps = []
    for c in range(NCORES):
        natc = np.ascontiguousarray(np.roll(xh, -c * RPC, axis=0))
        mtc = np.ascontiguousarray(natc.T)
        in_maps.append({"mt": mtc, "nat": natc, "eye": eye})
    return in_maps


def kernel(mapping: np.ndarray, **_kwargs) -> np.ndarray:
    mapping = np.asarray(mapping, dtype=np.float32)
    assert mapping.shape == (N, D)
    in_maps = prep_inputs(mapping)

    nc = _get_nc()
    res = run_bass_kernel_spmd(nc, in_maps, core_ids=list(range(NCORES)))

    out = np.empty((N, N), dtype=np.float32)
    for c in range(NCORES):
        out[c * RPC:(c + 1) * RPC] = np.roll(
            res.results[c]["out"].astype(np.float32), c * RPC, axis=1)
    return out


if __name__ == "__main__":
    rng = np.random.default_rng(0)
    x = rng.standard_normal((N, D)).astype(np.float32)
    o = kernel(mapping=x)
    print("out", o.shape, o.dtype, "sample", o[0, :4], "diag", np.abs(np.diag(o)).max())



# revision 5
# speedup vs baseline: 1.3971x; 1.2446x over previous
"""Pairwise Euclidean distance kernel for Trainium2 (8 NeuronCores, SPMD).

Problem: mapping [8192, 256] f32 -> out [8192, 8192] f32 where
out[i, j] = ||mapping[i] - mapping[j]||_2, via the GEMM identity
d2 = ||x_i||^2 + ||x_j||^2 - 2 <x_i, x_j>.

V2 (symmetric/triangle scheme): out is symmetric, so each core computes only
~56% of its row block and the host mirrors the rest. Core c (rotated so its
own 1024 rows sit first) computes, for each 512-row half h in {0, 1}, the
columns [h*512, h*512 + 4608) of its rotated tile: every 512-row "unit" a
covers column units a..a+8 (mod 16), so every unit pair (a, b) is covered
directly (offset <= 8) or via the transpose of (b, a) (offset >= 8, i.e.
16-offset <= 7). Output is stored f16 (rel err ~5e-4, well inside the 2e-2
gate) and widened on the host.

Per-core on-device pipeline (cost model ~47 us):
  - inputs: mt [256, 5120] f16 (rotated x^T, columns 0..5120), nat
    [5120, 256] f16 (same rows natural layout, for the row-sum squares),
    eye [128, 128] f32 (transpose identity).
  - 10 column units of 512; 10 row-sum groups of 512 rows each: squares on
    ACT (plain Square; fused/accum variants crash this HW), 3D-AP
    reduce_sum on DVE -> sqp_g [128, 4]; -0.5*sq split hi/lo into two f16
    rows (exact to ~2^-22) via PE transpose + strided flatten DMA ->
    sqf_g [2, 512].
  - main loop over 10 chunks: 8 chunks of [128, 1024] (unit pairs (0,1),
    (1,2), (2,3), ... alternating halves) + 2 single-unit tails, x 4 row
    tiles each; K=256 f16 matmuls accumulate the gram in PSUM, a K=2
    rank-1 matmul with an all-ones stationary folds in -0.5*sq_j, ACT does
    the whole epilogue sqrt(-2*psum + sq_i) with per-partition bias,
    writing f16; diagonal [128, 128] blocks get a tensor_scalar_min clamp
    (psum <= 0.5*sq_i) first so ACT Sqrt never sees negatives.
  - schedule: groups 0/1 sq chains first at high priority, PE runway
    prefill of the first two row tiles, then chunk-ordered stream with sq
    chains and loads prefetched two chunks ahead.

Hardware pitfalls (this container's TRN2 + neuronxcc build):
  - InstTensorTensorReduce (fused DVE square+reduce) and ACT Square with
    accum_out both crash the device (NRT_EXEC_UNIT_UNRECOVERABLE); use
    plain Square + separate reduce_sum instead.
  - ACT Sqrt on negative inputs yields NaN (CoreSim asserts); clamp first.
"""

import sys

try:
    import concourse.bass as _probe  # noqa: F401
except ImportError:
    sys.path.insert(0, "/opt/trn_rl_repo")

import numpy as np

import concourse.bacc as bacc
import concourse.mybir as mybir
from concourse import tile
from concourse.bass_utils import run_bass_kernel_spmd

N = 8192          # number of points
D = 256           # feature dim
NCORES = 8
RPC = N // NCORES  # 1024 rows per core
U = 512            # unit = 512 rows/cols
SPAN_U = 9         # column units covered per 512-row half
NCOL = 10 * U      # columns of mt/nat each core holds (5120)
NG = 10            # sq groups of 512 rows each
NUNITS = N // U    # 16 global units

F16 = mybir.dt.float16
F32 = mybir.dt.float32

# chunk schedule: (half, (units...)) — 1024-wide pairs then 512 tails,
# ordered so chunk i only needs mt units <= i+1 and sq groups <= i+1
CHUNKS = [
    (0, (0, 1)), (1, (1, 2)), (0, (2, 3)), (1, (3, 4)),
    (0, (4, 5)), (1, (5, 6)), (0, (6, 7)), (1, (7, 8)),
    (0, (8,)), (1, (9,)),
]


def _build_nc(repeats=1, loop_n=None, stage_bufs=4, work_bufs=2):
    nc = bacc.Bacc(None, target_bir_lowering=False)
    mt_d = nc.dram_tensor("mt", [D, NCOL], F16, kind="ExternalInput")
    nat_d = nc.dram_tensor("nat", [NCOL, D], F16, kind="ExternalInput")
    eye_d = nc.dram_tensor("eye", [128, 128], F32, kind="ExternalInput")
    out_d = nc.dram_tensor("out", [RPC, NCOL], F16, kind="ExternalOutput")

    with tile.TileContext(nc) as tc:
        with (
            tc.tile_pool(name="big", bufs=1) as big,
            tc.tile_pool(name="work", bufs=work_bufs) as work,
            tc.tile_pool(name="stage", bufs=stage_bufs) as stage_pool,
            tc.tile_pool(name="ps", bufs=4, space="PSUM") as psum,
        ):
            if loop_n is not None:
                with tc.For_i(0, loop_n, 1):
                    _emit_body(nc, tc, big, work, stage_pool, psum,
                               mt_d, nat_d, eye_d, out_d)
            else:
                for _rep in range(repeats):
                    _emit_body(nc, tc, big, work, stage_pool, psum,
                               mt_d, nat_d, eye_d, out_d)

    nc.compile()
    return nc


def _emit_body(nc, tc, big, work, stage_pool, psum, mt_d, nat_d, eye_d, out_d):
    mt0 = big.tile([128, NCOL], F16, tag="mt0")
    mt1 = big.tile([128, NCOL], F16, tag="mt1")
    eye = big.tile([128, 128], F32, tag="eye")
    ones2 = big.tile([2, 128], F16, tag="ones2")
    # per-group sq tensors: a single shared tile would create false WAR/RAW
    # couplings (later groups write other slices while chunks read their
    # bias / rank-1 rows), serializing the pipeline
    sqp = []
    sqf = []
    for _g in range(NG):
        sqp_t = big.tile([128, 4], F32, tag=f"sqp{_g}")
        sqp.append(sqp_t)
        sqf_t = big.tile([2, U], F16, tag=f"sqf{_g}")
        sqf.append(sqf_t)
    half_own = big.tile([128, 8], F32, tag="half_own")

    nat_g = nat_d.rearrange("(g t p) d -> g p t d", g=NG, p=128)
    gts = {}
    for _g in range(NG):
        gt_slot = big.tile([128, 4, 256], F16, tag=f"natg{_g}")
        gts[_g] = gt_slot

    # initial loads: groups 0/1 + mt units 0..2 (chunk 0 needs units 0,1;
    # chunk 1 needs 2)
    nc.sync.dma_start(gts[0][:], nat_g[0])
    nc.sync.dma_start(gts[1][:], nat_g[1])
    nc.sync.dma_start(gts[2][:], nat_g[2])
    nc.sync.dma_start(mt0[:, 0:3 * U], mt_d[0:128, 0:3 * U])
    nc.sync.dma_start(mt1[:, 0:3 * U], mt_d[128:256, 0:3 * U])
    nc.sync.dma_start(eye[:], eye_d[:])

    def emit_loads(ci):
        # during chunk ci: fetch sq group ci+3 and mt unit ci+3 (chunk i
        # needs units/groups <= i+1, chains for group g are emitted during
        # chunk g-2)
        g = ci + 3
        if g < NG:
            nc.sync.dma_start(gts[g][:], nat_g[g])
            j = g * U
            nc.sync.dma_start(mt0[:, j:j + U], mt_d[0:128, j:j + U])
            nc.sync.dma_start(mt1[:, j:j + U], mt_d[128:256, j:j + U])

    def emit_sq_reduce(g):
        gt = gts[g]
        # square on ACT (plain Square — fused/accum variants crash this HW),
        # reduce on DVE: splits sq work across engines
        msq = work.tile([128, 4, 256], F32, tag="msq")
        nc.scalar.activation(msq[:], gt[:],
                             mybir.ActivationFunctionType.Square)
        nc.vector.reduce_sum(sqp[g][:, 0:4].unsqueeze(2), msq[:],
                             axis=mybir.AxisListType.X)
        mh32 = work.tile([128, 4], F32, tag=f"mh32_{g}")
        nc.vector.tensor_scalar_mul(mh32[:], sqp[g][:, 0:4], -0.5)
        hi16 = work.tile([128, 4], F16, tag=f"hi16_{g}")
        nc.vector.tensor_copy(hi16[:], mh32[:])
        hi32 = work.tile([128, 4], F32, tag=f"hi32_{g}")
        nc.vector.tensor_copy(hi32[:], hi16[:])
        lo32 = work.tile([128, 4], F32, tag=f"lo32_{g}")
        nc.vector.tensor_sub(lo32[:], mh32[:], hi32[:])
        if g < 2:
            # own-rows 0.5*sq_i for the diagonal clamp
            nc.vector.tensor_scalar_mul(half_own[:, g * 4:(g + 1) * 4],
                                        sqp[g][:, 0:4], 0.5)
        return mh32, lo32

    def emit_sq_flatten(g, mh32, lo32):
        # PE-transpose [128, 4] -> [4, 128], flatten to sqf_g rows; kept
        # separate so the in-order PE meets these only after the DVE chain
        # has had time to produce mh32/lo32
        for row, src in ((0, mh32), (1, lo32)):
            pt = psum.tile([4, 128], F32, tag="ps")
            nc.tensor.transpose(pt[:], src[:], eye[:])
            st = work.tile([4, 128], F16, tag="sqT")
            nc.vector.tensor_copy(st[:], pt[:])
            nc.sync.dma_start(
                sqf[g][row:row + 1, :].rearrange("o (t i) -> o t i", t=4),
                st[:],
            )

    nc.vector.memset(ones2[:], 1.0)
    # group 0/1 chains first at high priority (the first rank-1 blocks the
    # in-order PE until sqf lands)
    with tc.high_priority():
        c0 = emit_sq_reduce(0)
        c1 = emit_sq_reduce(1)

    def emit_kmms(ps, r, units):
        lhs0 = mt0[:, r * 128:(r + 1) * 128]
        lhs1 = mt1[:, r * 128:(r + 1) * 128]
        for s, u in enumerate(units):
            o = ps[:, s * U:(s + 1) * U]
            j = u * U
            nc.tensor.matmul(o, lhs0, mt0[:, j:j + U], start=True, stop=False)
            nc.tensor.matmul(o, lhs1, mt1[:, j:j + U], start=False, stop=False)

    def emit_rank1(ps, units):
        for s, u in enumerate(units):
            o = ps[:, s * U:(s + 1) * U]
            nc.tensor.matmul(o, ones2[:], sqf[u][:, :], start=False, stop=True)

    def emit_tail(ps, r, units):
        w = len(units) * U
        out_t = stage_pool.tile([128, 1024], F16, tag="stage")
        bias = sqp[r // 4][:, r % 4:r % 4 + 1]
        if r // 4 in units:
            # diagonal block: d2 can go negative from fp rounding and ACT
            # Sqrt needs inputs >= 0; clamp psum <= 0.5*sq_i there
            s = units.index(r // 4)
            off = s * U + (r % 4) * 128
            dg = ps[:, off:off + 128]
            nc.vector.tensor_scalar_min(dg, dg, half_own[:, r:r + 1])
        nc.scalar.activation(
            out_t[:, 0:w], ps[:, 0:w],
            mybir.ActivationFunctionType.Sqrt,
            bias=bias, scale=-2.0,
        )
        nc.sync.dma_start(
            out_d[r * 128:(r + 1) * 128,
                  units[0] * U:units[0] * U + w],
            out_t[:, 0:w],
        )

    for ci, (h, units) in enumerate(CHUNKS):
        rows = [4 * h + k for k in range(4)]
        if ci == 0:
            # runway: fill two psum slots with sq-independent k-matmuls so
            # the in-order PE streams while the sq chains complete
            ps0 = psum.tile([128, 1024], F32, tag="ps")
            emit_kmms(ps0, rows[0], units)
            ps1 = psum.tile([128, 1024], F32, tag="ps")
            emit_kmms(ps1, rows[1], units)
            emit_sq_flatten(0, *c0)
            emit_sq_flatten(1, *c1)
            emit_loads(ci)
            emit_rank1(ps0, units)
            emit_tail(ps0, rows[0], units)
            emit_rank1(ps1, units)
            emit_tail(ps1, rows[1], units)
            nxt = emit_sq_reduce(2)
            for r in rows[2:]:
                ps = psum.tile([128, 1024], F32, tag="ps")
                emit_kmms(ps, r, units)
                emit_rank1(ps, units)
                emit_tail(ps, r, units)
            emit_sq_flatten(2, *nxt)
            continue
        nxt = None
        g = ci + 2
        for idx, r in enumerate(rows):
            if idx == 0:
                emit_loads(ci)
                if g < NG:
                    nxt = emit_sq_reduce(g)
            if idx == 2 and nxt is not None:
                emit_sq_flatten(g, *nxt)
            ps = psum.tile([128, 1024], F32, tag="ps")
            emit_kmms(ps, r, units)
            emit_rank1(ps, units)
            emit_tail(ps, r, units)


_NC_CACHE = None


def _get_nc():
    global _NC_CACHE
    if _NC_CACHE is None:
        _NC_CACHE = _build_nc()
    return _NC_CACHE


def prep_inputs(mapping: np.ndarray) -> list:
    xh = mapping.astype(np.float16)
    eye = np.eye(128, dtype=np.float32)
    in_maps = []
    for c in range(NCORES):
        rot = np.roll(xh, -c * RPC, axis=0)
        natc = np.ascontiguousarray(rot[0:NCOL])
        mtc = np.ascontiguousarray(natc.T)
        in_maps.append({"mt": mtc, "nat": natc, "eye": eye})
    return in_maps


def kernel(mapping: np.ndarray, **_kwargs) -> np.ndarray:
    mapping = np.asarray(mapping, dtype=np.float32)
    assert mapping.shape == (N, D)
    in_maps = prep_inputs(mapping)

    nc = _get_nc()
    res = run_bass_kernel_spmd(nc, in_maps, core_ids=list(range(NCORES)))

    out = np.empty((N, N), dtype=np.float32)
    covered = np.zeros((NUNITS, NUNITS), dtype=bool)
    span = SPAN_U * U
    for c in range(NCORES):
        oc = res.results[c]["out"]  # [1024, 5120] f16
        for h in (0, 1):
            au = c * 2 + h
            block = oc[h * U:(h + 1) * U, h * U:h * U + span].astype(np.float32)
            gr0 = c * RPC + h * U
            gc0 = (c * RPC + h * U) % N
            first = min(span, N - gc0)
            out[gr0:gr0 + U, gc0:gc0 + first] = block[:, :first]
            if first < span:
                out[gr0:gr0 + U, 0:span - first] = block[:, first:]
            for cu in range(SPAN_U):
                covered[au, (au + cu) % NUNITS] = True
    for a in range(NUNITS):
        for b in range(NUNITS):
            if not covered[a, b]:
                out[a * U:(a + 1) * U, b * U:(b + 1) * U] = \
                    out[b * U:(b + 1) * U, a * U:(a + 1) * U].T
    return out


if __name__ == "__main__":
    rng = np.random.default_rng(0)
    x = rng.standard_normal((N, D)).astype(np.float32)
    o = kernel(mapping=x)
    sq = (x * x).sum(1)
    ref = np.sqrt(np.maximum(sq[:, None] + sq[None, :] - 2 * x @ x.T, 0))
    d = np.abs(o - ref)
    print("out", o.shape, o.dtype, "absmax diff", d.max(),
          "diag", np.abs(np.diag(o)).max())


# revision 6
# speedup vs baseline: 1.6888x; 1.2088x over previous
"""Pairwise Euclidean distance kernel for Trainium2 (8 NeuronCores, SPMD).

Problem: mapping [8192, 256] f32 -> out [8192, 8192] f32 where
out[i, j] = ||mapping[i] - mapping[j]||_2, via the GEMM identity
d2 = ||x_i||^2 + ||x_j||^2 - 2 <x_i, x_j>.

V3 = V2 (symmetric/triangle, f16 output) + sequencer/overhead engineering.
V2's trace showed PE.SEQ 100% busy (Matmult 42us exec + Ldweights 24us +
sems 12us), SP.SEQ 73us issuing 87 DMAs, HWDGE 54us of per-DMA fixed cost,
ACT 51us. V3:
  - [128, 1536] PSUM chunks (3 banks x 2 bufs + a separate 2-bank ring for
    the sq transposes): 24 uniform chunk-rows, no ragged 512 tails; 24
    epilogue ACT ops and 24 output DMAs instead of 40 each.
  - matmuls grouped by stationary operand (lhs0 x3 subs, lhs1 x3, ones x3)
    so post-schedule legalization skips repeated Ldweights.
  - sq hi/lo flattened with ONE PE transpose per group ([128, 8] ->
    [8, 128], hi in cols 0:4, lo in 4:8) and ONE strided DMA per group.
  - input DMAs merged: mt in 1024/2048-col slices (8), nat in 1024-row
    pairs (5).
  - output stores issued from the otherwise-idle Pool engine (SWDGE path),
    off the SP sequencer and the shared HWDGE unit.

Scheme recap: core c is rotated so its own 1024 rows sit first; for each
512-row half h it computes columns [h*512, h*512+4608) of its rotated tile
(unit a covers column units a..a+8 mod 16; every pair is covered directly
or by the transpose of its mirror; the host mirrors the remaining 112
blocks). Output f16 (rel err ~5e-4 vs the 2e-2 gate), widened on the host.

Hardware pitfalls (this container's TRN2 + neuronxcc build):
  - InstTensorTensorReduce (fused DVE square+reduce) and ACT Square with
    accum_out both crash the device (NRT_EXEC_UNIT_UNRECOVERABLE); use
    plain Square + separate reduce_sum instead.
  - ACT Sqrt on negative inputs yields NaN (CoreSim asserts); clamp first.
"""

import sys

try:
    import concourse.bass as _probe  # noqa: F401
except ImportError:
    sys.path.insert(0, "/opt/trn_rl_repo")

import numpy as np

import concourse.bacc as bacc
import concourse.mybir as mybir
from concourse import tile
from concourse.bass_utils import run_bass_kernel_spmd

N = 8192          # number of points
D = 256           # feature dim
NCORES = 8
RPC = N // NCORES  # 1024 rows per core
U = 512            # unit = 512 rows/cols
SPAN_U = 9         # column units covered per 512-row half
NCOL = 10 * U      # columns of mt/nat each core holds (5120)
NG = 10            # sq groups of 512 rows each
NPAIR = 5          # nat load pairs (1024 rows)
NUNITS = N // U    # 16 global units
CW = 3 * U         # chunk width 1536

F16 = mybir.dt.float16
F32 = mybir.dt.float32

# entry schedule: (half, (units...)) — 1536-wide, ordered so entry i's
# chains/loads are prefetched during earlier entries
ENTRIES = [
    (0, (0, 1, 2)), (1, (1, 2, 3)),
    (0, (3, 4, 5)), (1, (4, 5, 6)),
    (0, (6, 7, 8)), (1, (7, 8, 9)),
]
# sq chains to emit inside each entry (groups 0-2 run before entry 0)
CHAINS = {0: (3, 4), 1: (5, 6), 2: (7,), 3: (8,), 4: (9,)}


def _build_nc(repeats=1, loop_n=None, stage_bufs=4, work_bufs=2):
    nc = bacc.Bacc(None, target_bir_lowering=False)
    mt_d = nc.dram_tensor("mt", [D, NCOL], F16, kind="ExternalInput")
    nat_d = nc.dram_tensor("nat", [NCOL, D], F16, kind="ExternalInput")
    eye_d = nc.dram_tensor("eye", [128, 128], F32, kind="ExternalInput")
    out_d = nc.dram_tensor("out", [RPC, NCOL], F16, kind="ExternalOutput")

    with tile.TileContext(nc) as tc:
        with (
            tc.tile_pool(name="big", bufs=1) as big,
            tc.tile_pool(name="work", bufs=work_bufs) as work,
            tc.tile_pool(name="stage", bufs=stage_bufs) as stage_pool,
            tc.tile_pool(name="ps", bufs=2, space="PSUM") as psum,
        ):
            if loop_n is not None:
                with tc.For_i(0, loop_n, 1):
                    _emit_body(nc, tc, big, work, stage_pool, psum,
                               mt_d, nat_d, eye_d, out_d)
            else:
                for _rep in range(repeats):
                    _emit_body(nc, tc, big, work, stage_pool, psum,
                               mt_d, nat_d, eye_d, out_d)

    nc.compile()
    return nc


def _emit_body(nc, tc, big, work, stage_pool, psum, mt_d, nat_d, eye_d, out_d):
    mt0 = big.tile([128, NCOL], F16, tag="mt0")
    mt1 = big.tile([128, NCOL], F16, tag="mt1")
    eye = big.tile([128, 128], F32, tag="eye")
    ones2 = big.tile([2, 128], F16, tag="ones2")
    # per-group sq tensors: a shared tile would create false WAR/RAW
    # couplings, serializing the pipeline
    sqp = []
    sqf = []
    for _g in range(NG):
        sqp_t = big.tile([128, 4], F32, tag=f"sqp{_g}")
        sqp.append(sqp_t)
        sqf_t = big.tile([2, U], F16, tag=f"sqf{_g}")
        sqf.append(sqf_t)
    half_own = big.tile([128, 8], F32, tag="half_own")

    natp = nat_d.rearrange("(q t p) d -> q p t d", q=NPAIR, p=128)
    gtp = {}
    for _q in range(NPAIR):
        gt_slot = big.tile([128, 8, 256], F16, tag=f"natp{_q}")
        gtp[_q] = gt_slot

    # initial loads: nat pairs 0-1 (sq groups 0..3) + mt units 0..3
    nc.sync.dma_start(gtp[0][:], natp[0])
    nc.sync.dma_start(gtp[1][:], natp[1])
    nc.sync.dma_start(mt0[:, 0:2048], mt_d[0:128, 0:2048])
    nc.sync.dma_start(mt1[:, 0:2048], mt_d[128:256, 0:2048])
    nc.sync.dma_start(eye[:], eye_d[:])

    def emit_loads(ei):
        if ei == 0:
            nc.sync.dma_start(gtp[2][:], natp[2])
            nc.sync.dma_start(mt0[:, 2048:3072], mt_d[0:128, 2048:3072])
            nc.sync.dma_start(mt1[:, 2048:3072], mt_d[128:256, 2048:3072])
        elif ei == 1:
            nc.sync.dma_start(gtp[3][:], natp[3])
            nc.sync.dma_start(mt0[:, 3072:4096], mt_d[0:128, 3072:4096])
            nc.sync.dma_start(mt1[:, 3072:4096], mt_d[128:256, 3072:4096])
        elif ei == 2:
            nc.sync.dma_start(gtp[4][:], natp[4])
            nc.sync.dma_start(mt0[:, 4096:NCOL], mt_d[0:128, 4096:NCOL])
            nc.sync.dma_start(mt1[:, 4096:NCOL], mt_d[128:256, 4096:NCOL])

    def emit_sq_reduce(g):
        gt = gtp[g // 2][:, (g % 2) * 4:(g % 2) * 4 + 4, :]
        # square on ACT (plain Square — fused/accum variants crash this HW),
        # reduce on DVE
        msq = work.tile([128, 4, 256], F32, tag="msq")
        nc.scalar.activation(msq[:], gt,
                             mybir.ActivationFunctionType.Square)
        nc.vector.reduce_sum(sqp[g][:, 0:4].unsqueeze(2), msq[:],
                             axis=mybir.AxisListType.X)
        # -0.5*sq split hi/lo (exact to ~2^-22): hi source in cols 0:4,
        # f16 residual in cols 4:8 of one tile so a single PE transpose
        # flattens both rows
        mhl = work.tile([128, 8], F32, tag=f"mhl{g}")
        nc.vector.tensor_scalar_mul(mhl[:, 0:4], sqp[g][:, 0:4], -0.5)
        hi16 = work.tile([128, 4], F16, tag="hi16")
        nc.vector.tensor_copy(hi16[:], mhl[:, 0:4])
        hi32 = work.tile([128, 4], F32, tag="hi32")
        nc.vector.tensor_copy(hi32[:], hi16[:])
        nc.vector.tensor_sub(mhl[:, 4:8], mhl[:, 0:4], hi32[:])
        if g < 2:
            # own-rows 0.5*sq_i for the diagonal clamp
            nc.vector.tensor_scalar_mul(half_own[:, g * 4:(g + 1) * 4],
                                        sqp[g][:, 0:4], 0.5)
        return mhl

    def emit_sq_flatten(g, mhl):
        # one PE transpose [128, 8] -> [8, 128] (partition 4r+t holds row
        # r's tile-t slab), one f16 copy, one flatten DMA
        pt = psum.tile([8, 128], F32, tag="pst")
        nc.tensor.transpose(pt[:], mhl[:], eye[:])
        st = work.tile([8, 128], F16, tag="sqT")
        nc.vector.tensor_copy(st[:], pt[:])
        nc.sync.dma_start(
            sqf[g].rearrange("r (t i) -> r t i", t=4),
            st[:],
        )

    def emit_chain(g):
        emit_sq_flatten(g, emit_sq_reduce(g))

    def emit_kmms(ps, r, units):
        # grouped by stationary so legalization drops repeated Ldweights
        lhs0 = mt0[:, r * 128:(r + 1) * 128]
        lhs1 = mt1[:, r * 128:(r + 1) * 128]
        for s, u in enumerate(units):
            j = u * U
            nc.tensor.matmul(ps[:, s * U:(s + 1) * U], lhs0,
                             mt0[:, j:j + U], start=True, stop=False)
        for s, u in enumerate(units):
            j = u * U
            nc.tensor.matmul(ps[:, s * U:(s + 1) * U], lhs1,
                             mt1[:, j:j + U], start=False, stop=False)

    def emit_rank1(ps, units):
        for s, u in enumerate(units):
            nc.tensor.matmul(ps[:, s * U:(s + 1) * U], ones2[:],
                             sqf[u][:, :], start=False, stop=True)

    def emit_tail(ps, r, units):
        out_t = stage_pool.tile([128, CW], F16, tag="stage")
        bias = sqp[r // 4][:, r % 4:r % 4 + 1]
        if r // 4 in units:
            # diagonal block: clamp psum <= 0.5*sq_i so ACT Sqrt input
            # -2*psum + sq_i stays >= 0 under fp rounding
            s = units.index(r // 4)
            off = s * U + (r % 4) * 128
            dg = ps[:, off:off + 128]
            nc.vector.tensor_scalar_min(dg, dg, half_own[:, r:r + 1])
        nc.scalar.activation(
            out_t[:], ps[:],
            mybir.ActivationFunctionType.Sqrt,
            bias=bias, scale=-2.0,
        )
        # store from the Pool engine: SWDGE path, keeps the 24 output DMAs
        # off the SP sequencer and the shared HWDGE unit
        nc.gpsimd.dma_start(
            out_d[r * 128:(r + 1) * 128,
                  units[0] * U:units[0] * U + CW],
            out_t[:],
        )

    nc.vector.memset(ones2[:], 1.0)
    with tc.high_priority():
        emit_chain(0)
        emit_chain(1)

    for ei, (h, units) in enumerate(ENTRIES):
        rows = [4 * h + k for k in range(4)]
        chain = CHAINS.get(ei, ())
        if ei == 0:
            # runway: slot 0's k-matmuls stream while the group-2 chain
            # completes; rank-1s join once sqf lands
            ps0 = psum.tile([128, CW], F32, tag="ps")
            emit_kmms(ps0, rows[0], units)
            with tc.high_priority():
                emit_chain(2)
            emit_loads(0)
            ps1 = psum.tile([128, CW], F32, tag="ps")
            emit_kmms(ps1, rows[1], units)
            emit_rank1(ps0, units)
            emit_tail(ps0, rows[0], units)
            emit_rank1(ps1, units)
            emit_tail(ps1, rows[1], units)
            for idx, r in enumerate(rows[2:]):
                if idx < len(chain):
                    emit_chain(chain[idx])
                ps = psum.tile([128, CW], F32, tag="ps")
                emit_kmms(ps, r, units)
                emit_rank1(ps, units)
                emit_tail(ps, r, units)
            continue
        for idx, r in enumerate(rows):
            if idx == 0:
                emit_loads(ei)
            if idx in (1, 3) and len(chain) > idx // 2:
                emit_chain(chain[idx // 2])
            ps = psum.tile([128, CW], F32, tag="ps")
            emit_kmms(ps, r, units)
            emit_rank1(ps, units)
            emit_tail(ps, r, units)


_NC_CACHE = None


def _get_nc():
    global _NC_CACHE
    if _NC_CACHE is None:
        _NC_CACHE = _build_nc()
    return _NC_CACHE


def prep_inputs(mapping: np.ndarray) -> list:
    xh = mapping.astype(np.float16)
    eye = np.eye(128, dtype=np.float32)
    in_maps = []
    for c in range(NCORES):
        rot = np.roll(xh, -c * RPC, axis=0)
        natc = np.ascontiguousarray(rot[0:NCOL])
        mtc = np.ascontiguousarray(natc.T)
        in_maps.append({"mt": mtc, "nat": natc, "eye": eye})
    return in_maps


def kernel(mapping: np.ndarray, **_kwargs) -> np.ndarray:
    mapping = np.asarray(mapping, dtype=np.float32)
    assert mapping.shape == (N, D)
    in_maps = prep_inputs(mapping)

    nc = _get_nc()
    res = run_bass_kernel_spmd(nc, in_maps, core_ids=list(range(NCORES)))

    out = np.empty((N, N), dtype=np.float32)
    covered = np.zeros((NUNITS, NUNITS), dtype=bool)
    span = SPAN_U * U
    for c in range(NCORES):
        oc = res.results[c]["out"]  # [1024, 5120] f16
        for h in (0, 1):
            au = c * 2 + h
            block = oc[h * U:(h + 1) * U, h * U:h * U + span].astype(np.float32)
            gr0 = c * RPC + h * U
            gc0 = (c * RPC + h * U) % N
            first = min(span, N - gc0)
            out[gr0:gr0 + U, gc0:gc0 + first] = block[:, :first]
            if first < span:
                out[gr0:gr0 + U, 0:span - first] = block[:, first:]
            for cu in range(SPAN_U):
                covered[au, (au + cu) % NUNITS] = True
    for a in range(NUNITS):
        for b in range(NUNITS):
            if not covered[a, b]:
                out[a * U:(a + 1) * U, b * U:(b + 1) * U] = \
                    out[b * U:(b + 1) * U, a * U:(a + 1) * U].T
    return out


if __name__ == "__main__":
    rng = np.random.default_rng(0)
    x = rng.standard_normal((N, D)).astype(np.float32)
    o = kernel(mapping=x)
    sq = (x * x).sum(1)
    ref = np.sqrt(np.maximum(sq[:, None] + sq[None, :] - 2 * x @ x.T, 0))
    d = np.abs(o - ref)
    print("out", o.shape, o.dtype, "absmax diff", d.max(),
          "diag", np.abs(np.diag(o)).max())
